# revision 14
# baseline (speedup 1.0000x reference)
"""Trainium2 Bass kernel for AccumulativeGainLoss — fp8-stream version.

Data-parallel over B across 8 NeuronCores (JB=2 batch elements per core).

Math (same restructure as the bf16 baseline, validated on host):
    H    = [F|1]^T [F|1]      bf16 PE, PSUM accum         [33,33]
    inv  = (F^T F)^{-1}       Newton-Schulz 3 iters
    M;sumy = [F|m]^T Y        fp8 DoubleRow PE stream     [64,256]
    sy2  = mask^T Y^2         bf16 PE reduce of squares
    q    = colsum(M * inv M);  ss_res = sy2 - q
    ss_tot = sy2 - sumy^2/N + EPS;  r2 = 1 - ss_res/ss_tot
    wsum = sum(w * r2);  cov = FtF - s s^T/N; quad = c^T (cov*cov) c
loss = mean_b(-wsum/T) + 0.1 * mean_b(quad - K)

Design (v4):
- Y ships as fp8e4m3 (3.1MB/core vs 6.2 bf16); M/sumy stream runs as
  DoubleRow fp8 matmuls (K=256 per matmul, 256 cyc on HW).  Host sim:
  rel err ~7.6e-4 (Y^2 must stay bf16; fp8 Y^2 costs 3e-3).
- Two parallel HWDGE DMA rings (sync=j0, scalar=j1), each depth-2
  chained, Y in 20/16/12-chunk blocks (descending so the last block's
  square+reduce tail is short); PE consumes blocks j-interleaved.
- Y^2 = square(fp8 Y) -> bf16 split ACT/DVE/Pool ~ (11:8:1)/20 per
  block (HW-measured rates; Pool runs fp8 mults at ~0.15 eff);
  partition-reduced by bf16 matmuls (mask col -> sy2 row).
- Batched epilogue: j0's sumy/sy2/q land on partition 32, j1's on 63
  (mask at f8-image col 32/63, q via a ones-at-63 lhsT), so one
  [32:64]-partition chain — single reciprocal — serves both js.  The
  two partial w-sums ship to DRAM rows 0/31; host adds them.
- PSUM: GS{j} (DR out + warmup), Y2S{j}, big{j} (H/P/q via tag
  rotation), tns x2 = exactly 8 banks.
"""

import ml_dtypes
import numpy as np

import concourse.bacc as bacc
import concourse.bass as bass
import concourse.mybir as mybir
import concourse.tile as tile
from concourse.bass_utils import run_bass_kernel_spmd
from concourse.tile_rust import add_dep_helper

F32 = mybir.dt.float32
F32R = mybir.dt.float32r
BF16 = mybir.dt.bfloat16
F8 = mybir.dt.float8e4
ALU = mybir.AluOpType
AX = mybir.AxisListType
DR = mybir.MatmulPerfMode.DoubleRow

B, T, N, K, D = 16, 32, 6000, 32, 8
NCORES = 8
JB = B // NCORES          # batch elements per core
NCH = 47                  # ceil(6000/128) real chunks of 128 rows
NCHP = 48                 # padded chunk count (DR pairing)
PAIRS = NCHP // 2         # 24 DoubleRow pair-matmuls per j
TD = T * D                # 256
FW = 34                   # f16 image: 32 coeffs + mask + pad
FROW = NCH * FW           # 1598
KS = 64                   # f8 k-tile stride: dual-fp8 ldweights needs the
                          # outer weight step even and 16B-aligned; 64 also
                          # puts j1's sumy on out partition 63 (mask col 63)
PW = 2 * KS               # f8 pair stride
F8ROW = PAIRS * PW        # 3072
YROW = NCHP * TD          # 12288
BLOCKS = (12, 16, 20)     # chunks per Y block (ascending: first arrives fast)
NBLK = len(BLOCKS)
BOFF = (0, 12, 28)        # chunk offset of each block
# squares ACT/DVE chunks per (j, block); Pool measured ~2us/chunk on fp8
# so it carries the NS/epilogue chains instead.  Last blocks split
# asymmetrically so j0's Y2 can start while j1's squares still run.
SQ_SPLIT = {(0, 0): (7, 5), (1, 0): (7, 5),
            (0, 1): (9, 7), (1, 1): (9, 7),
            (0, 2): (13, 7), (1, 2): (7, 13)}
NWARM = 16                # PE p-state warmup matmuls
NS_ITERS = 3
EPS = 1e-8
DECAY = 0.9
PEN = 0.1

_CACHE = {}


def _build_program():
    nc = bacc.Bacc("TRN2", target_bir_lowering=False, debug=False)
    y_d = nc.declare_dram_parameter("y", [JB, 128, YROW], F8, isOutput=False)
    f_d = nc.declare_dram_parameter("f", [JB, 128, FROW], BF16, isOutput=False)
    g_d = nc.declare_dram_parameter("g", [JB, 128, F8ROW], F8, isOutput=False)
    c_d = nc.declare_dram_parameter("c32", [32, 160], F32, isOutput=False)
    w_d = nc.declare_dram_parameter("w2", [32, TD], F32, isOutput=False)
    m_d = nc.declare_dram_parameter("mk", [128, 1], BF16, isOutput=False)
    o_d = nc.declare_dram_parameter("out", [32, 2], F32, isOutput=True)

    with tile.TileContext(nc) as tc:
        with (
            tc.tile_pool(name="cpool", bufs=1) as cpool,
            tc.tile_pool(name="fpool", bufs=1) as fpool,
            tc.tile_pool(name="ypool", bufs=1) as ypool,
            tc.tile_pool(name="y2pool", bufs=1) as y2pool,
            tc.tile_pool(name="nsb", bufs=2) as nsb,
            tc.tile_pool(name="esb", bufs=2) as esb,
            tc.tile_pool(name="ps", bufs=1, space="PSUM") as ps,
        ):
            # ---- PSUM banks (8 total): GS{j}, Y2S{j}, big{j}, tns x2
            GS = [ps.tile([64, 512], F32, tag=f"GS{j}", name=f"GS{j}")
                  for j in range(JB)]
            Y2S = [ps.tile([64, 512], F32, tag=f"Y2S{j}", name=f"Y2S{j}")
                   for j in range(JB)]

            # ---- PE warmup into the GS banks (overwritten by the real
            # DoubleRow groups, which re-start the accumulation).
            wtile = cpool.tile([128, 512], BF16)
            nc.vector.memset(wtile, 0.01)
            for i in range(NWARM):
                nc.tensor.matmul(GS[i % 2][0:64, 0:512], wtile[:, 0:64],
                                 wtile, start=True, stop=True)

            # ---- DMAs: ALL stream transfers on the sync ring (a trigger
            # whose chain-wait is pending stalls its whole sequencer, so
            # the ACT/Pool engines must carry no stream triggers or their
            # squares queue behind the waits).  Global depth-2 chain in
            # arrival-shaped order: first Y blocks, then F/G, then the
            # rest of Y.
            ftile = fpool.tile([128, JB * FROW], BF16)
            gtile = fpool.tile([128, JB * F8ROW], F8)
            ytiles = {}
            for b in range(NBLK):
                for j in range(JB):
                    ytiles[(j, b)] = ypool.tile(
                        [128, BLOCKS[b] * TD], F8,
                        tag=f"yb{j}_{b}", name=f"yb{j}_{b}")
            chain = []

            def chain_dma(out, in_):
                dma = nc.sync.dma_start(out=out, in_=in_)
                if len(chain) >= 3:
                    add_dep_helper(dma.ins, chain[-3].ins, sync=True,
                                   reason="depth-3 stream chain")
                chain.append(dma)
                return dma

            def y_dma(j, b):
                chain_dma(ytiles[(j, b)][:, :],
                          y_d[j, :, BOFF[b] * TD:(BOFF[b] + BLOCKS[b]) * TD])

            y_dma(0, 0)
            y_dma(1, 0)
            chain_dma(ftile[:, 0:FROW], f_d[0, :, :])
            chain_dma(gtile[:, 0:F8ROW], g_d[0, :, :])
            chain_dma(ftile[:, FROW:2 * FROW], f_d[1, :, :])
            chain_dma(gtile[:, F8ROW:2 * F8ROW], g_d[1, :, :])
            y_dma(0, 1)
            y_dma(1, 1)
            y_dma(0, 2)
            y_dma(1, 2)

            consts = cpool.tile([32, 160], F32)
            nc.gpsimd.dma_start(out=consts, in_=c_d[:, :])
            eye = consts[:, 0:32]
            twoI = consts[:, 32:64]
            ones2d = consts[:, 64:96]
            ones32 = consts[:, 64:65]
            ones64q = consts[:, 96:160]     # zeros with col 63 = 1

            # w2 on both epilogue partitions (32 for j0, 63 for j1):
            # host ships a [32, TD] image with rows 0 and 31 = w2 so one
            # DMA lands it on partitions 32..63 (SBUF APs may only start
            # at partition 0/32/64/96)
            w2sb = cpool.tile([64, TD], F32)
            nc.gpsimd.dma_start(out=w2sb[32:64, :], in_=w_d[:, :])
            sumw = cpool.tile([64, 1], F32)
            nc.vector.reduce_sum(sumw[32:64, :], w2sb[32:64, :], axis=AX.X)

            # combined epilogue staging: cols 0:256 sumy, 256:512 sy2,
            # 512:768 q; row 32 = j0, row 63 = j1
            comb = cpool.tile([64, 3 * TD], F32)
            nc.vector.memset(comb, 0.0)

            def fch(j, c):  # chunk-c [F|mask] block [128, 33] bf16
                return ftile[:, j * FROW + c * FW: j * FROW + c * FW + 33]

            # Y2-reduce lhsT tiles: mask-only columns (col 32 for j0,
            # col 63 for j1) so sy2 lands on the epilogue partitions and
            # the Y2 matmuls have no dependency on the F image at all.
            # mk = chunk-46 row-validity mask, shipped as its own tiny
            # param so the B variants are ready before F arrives.
            mksb = cpool.tile([128, 1], BF16)
            nc.gpsimd.dma_start(out=mksb, in_=m_d[:, :])
            m0A = cpool.tile([128, 33], BF16)
            nc.gpsimd.memset(m0A, 0.0)
            nc.gpsimd.memset(m0A[:, 32:33], 1.0)
            m0B = cpool.tile([128, 33], BF16)
            nc.gpsimd.memset(m0B, 0.0)
            nc.vector.tensor_copy(m0B[:, 32:33], mksb)
            m1A = cpool.tile([128, 64], BF16)
            nc.gpsimd.memset(m1A, 0.0)
            nc.gpsimd.memset(m1A[:, 63:64], 1.0)
            m1B = cpool.tile([128, 64], BF16)
            nc.gpsimd.memset(m1B, 0.0)
            nc.vector.tensor_copy(m1B[:, 63:64], mksb)

            # ---- H = [F|mask]^T [F|mask] per j (needs only the F image)
            Hsb_j = [None, None]

            def emit_H(j):
                Hps = ps.tile([64, 512], F32, tag=f"big{j}")
                for c in range(NCH):
                    nc.tensor.matmul(Hps[0:33, 0:33], fch(j, c), fch(j, c),
                                     start=(c == 0), stop=(c == NCH - 1))
                Hsb = nsb.tile([33, 33], F32, tag="Hsb")
                nc.vector.tensor_copy(Hsb, Hps[0:33, 0:33])
                Hsb_j[j] = Hsb

            # ---- Newton-Schulz + corr-penalty step closures (PE steps
            # interleaved into the stream so the PE FIFO never head-blocks
            # on their DVE inputs).
            inv_sb = [None, None]
            quad_sb = [None, None]

            def make_steps(j):
                state = {}

                def s_trace():
                    Hsb = Hsb_j[j]
                    A = state["A"] = Hsb[0:32, 0:32]
                    state["s_row"] = Hsb[32:33, 0:32]
                    dm = nsb.tile([32, 32], F32, tag="dm")
                    nc.gpsimd.tensor_mul(dm, A, eye)
                    dg = nsb.tile([32, 1], F32, tag="dg")
                    nc.vector.reduce_sum(dg, dm, axis=AX.X)
                    trp = ps.tile([128, 512], F32, tag="tns", bufs=2)
                    nc.tensor.matmul(trp[0:32, 0:1], ones2d, dg,
                                     start=True, stop=True)
                    rtr = nsb.tile([32, 1], F32, tag="rtr")
                    nc.vector.reciprocal(rtr, trp[0:32, 0:1])
                    c0v = nsb.tile([32, 1], F32, tag="c0v")
                    nc.gpsimd.tensor_scalar_mul(c0v, rtr, float(K))
                    X = nsb.tile([32, 32], F32, tag="Xns", bufs=2 * NS_ITERS + 4)
                    nc.vector.tensor_scalar(X, eye, c0v, None, ALU.mult)
                    state["X"] = X
                steps = [s_trace]

                def ns_a():
                    t1 = ps.tile([128, 512], F32, tag="tns", bufs=2)
                    t1 = t1[0:32, 0:32]
                    nc.tensor.matmul(t1, state["A"], state["X"],
                                     start=True, stop=True)
                    z = nsb.tile([32, 32], F32, tag="Zns",
                                 bufs=2 * NS_ITERS + 2)
                    nc.vector.tensor_sub(z, twoI, t1)
                    state["z"] = z

                def ns_b():
                    x2 = ps.tile([128, 512], F32, tag="tns", bufs=2)
                    x2 = x2[0:32, 0:32]
                    nc.tensor.matmul(x2, state["X"], state["z"],
                                     start=True, stop=True)
                    Xn = nsb.tile([32, 32], F32, tag="Xns",
                                  bufs=2 * NS_ITERS + 4)
                    nc.vector.tensor_copy(Xn, x2)
                    state["X"] = Xn
                for _ in range(NS_ITERS):
                    steps += [ns_a, ns_b]

                def c_outer():
                    inv_sb[j] = state["X"]
                    outp = ps.tile([128, 512], F32, tag="tns", bufs=2)
                    outp = outp[0:32, 0:32]
                    nc.tensor.matmul(outp, state["s_row"], state["s_row"],
                                     start=True, stop=True)
                    covn = nsb.tile([32, 32], F32, tag="covn")
                    nc.vector.tensor_scalar_mul(covn, outp, 1.0 / N)
                    cov = nsb.tile([32, 32], F32, tag="cov")
                    nc.gpsimd.tensor_sub(cov, state["A"], covn)
                    dm2 = nsb.tile([32, 32], F32, tag="dm2")
                    nc.gpsimd.tensor_mul(dm2, cov, eye)
                    dg2 = nsb.tile([32, 1], F32, tag="dg2")
                    nc.vector.reduce_sum(dg2, dm2, axis=AX.X)
                    cv = nsb.tile([32, 1], F32, tag="cv")
                    nc.vector.reciprocal(cv, dg2)
                    A2 = nsb.tile([32, 32], F32, tag="A2")
                    nc.gpsimd.tensor_mul(A2, cov, cov)
                    state["cv"] = cv
                    state["A2"] = A2

                def c_u():
                    ups = ps.tile([128, 512], F32, tag="tns", bufs=2)
                    nc.tensor.matmul(ups[0:32, 0:1], state["A2"], state["cv"],
                                     start=True, stop=True)
                    usb = nsb.tile([32, 1], F32, tag="usb")
                    nc.vector.tensor_copy(usb, ups[0:32, 0:1])
                    state["usb"] = usb

                def c_q():
                    qd = ps.tile([128, 512], F32, tag="tns", bufs=2)
                    nc.tensor.matmul(qd[32:33, 0:1], state["usb"], state["cv"],
                                     start=True, stop=True)
                    qsb = nsb.tile([33, 1], F32, tag="qsb")
                    nc.vector.tensor_copy(qsb[32:33, :], qd[32:33, 0:1])
                    quad_sb[j] = qsb
                steps += [c_outer, c_u, c_q]
                return steps

            pending = {0: make_steps(0), 1: make_steps(1)}

            y2tiles = {}

            def emit_squares(j, b):
                yt = ytiles[(j, b)]
                y2t = y2pool.tile([128, BLOCKS[b] * TD], BF16,
                                  tag=f"y2_{j}_{b}", name=f"y2_{j}_{b}")
                na, nd = SQ_SPLIT[(j, b)]
                a = na * TD
                nc.scalar.square(y2t[:, 0:a], yt[:, 0:a])
                nc.vector.tensor_mul(y2t[:, a:], yt[:, a:], yt[:, a:])
                y2tiles[(j, b)] = y2t

            def emit_dr(j, b, pop_steps=False):
                yt = ytiles[(j, b)]
                steps = pending[j]
                for p in range(BLOCKS[b] // 2):
                    P = BOFF[b] // 2 + p
                    lhsT = gtile[:, j * F8ROW + P * PW: j * F8ROW + (P + 1) * PW
                                 ].rearrange("p (two m) -> p two m", two=2)
                    rhs = yt[:, p * 2 * TD:(p + 1) * 2 * TD
                             ].rearrange("p (two f) -> p two f", two=2)
                    nc.tensor.matmul(GS[j][0:64, 0:256], lhsT, rhs,
                                     start=(P == 0), stop=(P == PAIRS - 1),
                                     perf_mode=DR)
                    if pop_steps and p % 2 == 1 and steps:
                        steps.pop(0)()

            def emit_y2mm(j, b, pop_steps=False):
                y2t = y2tiles[(j, b)]
                steps = pending[j]
                for lc in range(BLOCKS[b]):
                    c = BOFF[b] + lc
                    if c >= NCH:
                        continue
                    if j == 0:
                        lhsT = m0B if c == NCH - 1 else m0A
                        out = Y2S[0][0:33, 0:256]
                    else:
                        lhsT = m1B if c == NCH - 1 else m1A
                        out = Y2S[1][0:64, 0:256]
                    nc.tensor.matmul(out, lhsT,
                                     y2t[:, lc * TD:(lc + 1) * TD],
                                     start=(c == 0), stop=(c == NCH - 1))
                    if pop_steps and lc % 2 == 1 and steps:
                        steps.pop(0)()

            qps_j = [None, None]

            def phase1(j):
                """sy2-independent epilogue: P = inv M, W, q.  Runs right
                after DR(j) stops; comb staging happens in phase2."""
                Gsb = esb.tile([32, TD], F32, tag="Gsb")
                nc.vector.tensor_copy(Gsb, GS[j][0:32, 0:256])
                Pps = ps.tile([64, 512], F32, tag=f"big{j}")
                nc.tensor.matmul(Pps[0:32, 0:TD], inv_sb[j], Gsb,
                                 start=True, stop=True)
                W = esb.tile([32, TD], F32, tag="W")
                nc.vector.tensor_mul(W, Gsb, Pps[0:32, 0:TD])
                qps = ps.tile([64, 512], F32, tag=f"big{j}")
                if j == 0:
                    nc.tensor.matmul(qps[32:33, 0:TD], ones32, W,
                                     start=True, stop=True)
                else:
                    nc.tensor.matmul(qps[0:64, 0:TD], ones64q, W,
                                     start=True, stop=True)
                qps_j[j] = qps

            def phase2():
                """Batched tail for both js on partitions 32..63.  SBUF
                APs may only start at partitions 0/32/64/96, so j1's row
                63 is staged via [32:64] block copies (its rows 32..62
                are zeros/junk), then j0's row 32 overwrites."""
                R = slice(32, 64)
                nc.vector.tensor_copy(comb[R, 0:TD], GS[1][32:64, 0:256])
                nc.vector.tensor_copy(comb[32:33, 0:TD], GS[0][32:33, 0:256])
                nc.vector.tensor_copy(comb[R, TD:2 * TD],
                                      Y2S[1][32:64, 0:256])
                nc.vector.tensor_copy(comb[32:33, TD:2 * TD],
                                      Y2S[0][32:33, 0:256])
                nc.vector.tensor_copy(comb[R, 2 * TD:3 * TD],
                                      qps_j[1][32:64, 0:TD])
                nc.vector.tensor_copy(comb[32:33, 2 * TD:3 * TD],
                                      qps_j[0][32:33, 0:TD])
                sumy = comb[R, 0:TD]
                sy2 = comb[R, TD:2 * TD]
                qrow = comb[R, 2 * TD:3 * TD]
                sstot_a = esb.tile([64, TD], F32, tag="sstot_a")
                nc.vector.scalar_tensor_tensor(
                    sstot_a[R, :], sumy, -1.0 / N, sumy, ALU.mult, ALU.mult)
                sstot = esb.tile([64, TD], F32, tag="sstot")
                nc.vector.scalar_tensor_tensor(
                    sstot[R, :], sstot_a[R, :], EPS, sy2, ALU.add, ALU.add)
                rec = esb.tile([64, TD], F32, tag="rec")
                nc.vector.reciprocal(rec[R, :], sstot[R, :])
                wrec = esb.tile([64, TD], F32, tag="wrec")
                nc.vector.tensor_mul(wrec[R, :], rec[R, :], w2sb[R, :])
                tA = esb.tile([64, TD], F32, tag="tA")
                accA = esb.tile([64, 1], F32, tag="accA")
                nc.vector.scalar_tensor_tensor(
                    tA[R, :], sy2, 1.0, wrec[R, :],
                    ALU.mult, ALU.mult, accum_out=accA[R, :])
                tB = esb.tile([64, TD], F32, tag="tB")
                accB = esb.tile([64, 1], F32, tag="accB")
                nc.vector.scalar_tensor_tensor(
                    tB[R, :], qrow, 1.0, wrec[R, :],
                    ALU.mult, ALU.mult, accum_out=accB[R, :])
                d1 = esb.tile([64, 1], F32, tag="d1")
                nc.vector.tensor_sub(d1[R, :], sumw[R, :], accA[R, :])
                outsb = cpool.tile([64, 2], F32)
                nc.vector.memset(outsb, 0.0)
                nc.vector.tensor_add(outsb[R, 0:1], d1[R, :], accB[R, :])
                nc.vector.tensor_add(outsb[32:33, 1:2],
                                     quad_sb[0][32:33, :],
                                     quad_sb[1][32:33, :])
                nc.sync.dma_start(out=o_d[:, :], in_=outsb[R, 0:2])

            # ---- stream emission in DMA-arrival order (PE is in-order,
            # so a stalled instruction blocks everything behind it):
            # y(b0) -> Y2(b0);  f16 -> H;  g -> DR;  interleave NS.
            emit_squares(0, 0)
            emit_squares(1, 0)
            emit_y2mm(0, 0)
            emit_y2mm(1, 0)
            emit_H(0)
            emit_dr(0, 0, pop_steps=True)
            emit_H(1)
            emit_dr(1, 0, pop_steps=True)
            emit_squares(0, 1)
            emit_squares(1, 1)
            emit_y2mm(0, 1, pop_steps=True)
            emit_y2mm(1, 1, pop_steps=True)
            emit_dr(0, 1, pop_steps=True)
            emit_dr(1, 1, pop_steps=True)
            emit_squares(0, 2)
            emit_squares(1, 2)
            emit_dr(0, 2)
            phase1(0)
            emit_dr(1, 2)
            phase1(1)
            emit_y2mm(0, 2)
            emit_y2mm(1, 2)
            phase2()

    nc.compile()
    return nc


def _prepare_in_maps(preds, y_ts, importance):
    preds = np.ascontiguousarray(preds, dtype=np.float32)
    y_ts = np.ascontiguousarray(y_ts, dtype=np.float32)
    importance = np.ascontiguousarray(importance, dtype=np.float32)

    bf16 = ml_dtypes.bfloat16
    f8 = ml_dtypes.float8_e4m3fn
    NPAD = NCHP * 128     # 6144

    # Y image: yimg[b, p, c*TD + t*D + d] = fp8(y_ts[b, t, c*128+p, d])
    ypad = np.zeros((B, T, NPAD, D), dtype=f8)
    ypad[:, :, :N, :] = y_ts.astype(f8)
    yimg = np.ascontiguousarray(
        ypad.reshape(B, T, NCHP, 128, D).transpose(0, 3, 2, 1, 4)
    ).reshape(B, 128, YROW)

    # F bf16 image: fimg[b, p, c*FW + k]; col 32 = valid-mask
    fpad = np.zeros((B, NCH * 128, FW), dtype=bf16)
    fpad[:, :N, :K] = preds.astype(bf16)
    fpad[:, :N, K] = 1.0
    fimg = np.ascontiguousarray(
        fpad.reshape(B, NCH, 128, FW).transpose(0, 2, 1, 3)
    ).reshape(B, 128, FROW)

    # F fp8 image, 48 chunks, pair-major for DoubleRow lhsT; k-tile
    # stride KS=64; mask col 32 for even batch (j0), 63 for odd (j1)
    gpad = np.zeros((B, NPAD, KS), dtype=f8)
    gpad[:, :N, :K] = preds.astype(f8)
    gpad[0::2, :N, 32] = 1.0
    gpad[1::2, :N, 63] = 1.0
    gimg = np.ascontiguousarray(
        gpad.reshape(B, NCHP, 128, KS).transpose(0, 2, 1, 3)
    ).reshape(B, 128, F8ROW)

    c32 = np.zeros((32, 160), dtype=np.float32)
    c32[:, 0:32] = np.eye(32, dtype=np.float32)
    c32[:, 32:64] = 2.0 * np.eye(32, dtype=np.float32)
    c32[:, 64:96] = 1.0
    c32[:, 96 + 63] = 1.0

    decay = DECAY ** np.arange(T, dtype=np.float32)
    w2row = (decay[:, None] * importance[None, :].astype(np.float32)
             ).reshape(TD)
    w2 = np.zeros((32, TD), dtype=np.float32)
    w2[0] = w2row
    w2[31] = w2row

    mk = np.zeros((128, 1), dtype=bf16)
    mk[:N - (NCH - 1) * 128, 0] = 1.0

    in_maps = []
    for i in range(NCORES):
        in_maps.append({
            "y": np.ascontiguousarray(yimg[i * JB:(i + 1) * JB]),
            "f": np.ascontiguousarray(fimg[i * JB:(i + 1) * JB]),
            "g": np.ascontiguousarray(gimg[i * JB:(i + 1) * JB]),
            "c32": c32,
            "w2": w2,
            "mk": mk,
        })
    return in_maps


def _combine(results):
    loss = 0.0
    for r in results:
        w_total = float(r["out"][0, 0]) + float(r["out"][31, 0])
        q_total = float(r["out"][0, 1])
        loss += (-w_total / T + PEN * (q_total - JB * K)) / B
    return np.float32(loss)


def run_on_device(preds, y_ts, importance, trace=False, **spmd_kwargs):
    if "nc" not in _CACHE:
        _CACHE["nc"] = _build_program()
    nc = _CACHE["nc"]
    in_maps = _prepare_in_maps(preds, y_ts, importance)
    res = run_bass_kernel_spmd(
        nc, in_maps, list(range(NCORES)), trace=trace, **spmd_kwargs
    )
    return _combine(res.results), res


def kernel(preds, y_ts, importance):
    loss, _ = run_on_device(preds, y_ts, importance, trace=False)
    return loss


# revision 15
# speedup vs baseline: 1.0638x; 1.0638x over previous
"""Trainium2 Bass kernel for AccumulativeGainLoss — fp8-stream version.

Data-parallel over B across 8 NeuronCores (JB=2 batch elements per core).

Math (same restructure as the bf16 baseline, validated on host):
    H    = [F|1]^T [F|1]      bf16 PE, PSUM accum         [33,33]
    inv  = (F^T F)^{-1}       Newton-Schulz 3 iters
    M;sumy = [F|m]^T Y        fp8 DoubleRow PE stream     [64,256]
    sy2  = mask^T Y^2         bf16 PE reduce of squares
    q    = colsum(M * inv M);  ss_res = sy2 - q
    ss_tot = sy2 - sumy^2/N + EPS;  r2 = 1 - ss_res/ss_tot
    wsum = sum(w * r2);  cov = FtF - s s^T/N; quad = c^T (cov*cov) c
loss = mean_b(-wsum/T) + 0.1 * mean_b(quad - K)

Design (v4):
- Y ships as fp8e4m3 (3.1MB/core vs 6.2 bf16); M/sumy stream runs as
  DoubleRow fp8 matmuls (K=256 per matmul, 256 cyc on HW).  Host sim:
  rel err ~7.6e-4 (Y^2 must stay bf16; fp8 Y^2 costs 3e-3).
- Two parallel HWDGE DMA rings (sync=j0, scalar=j1), each depth-2
  chained, Y in 20/16/12-chunk blocks (descending so the last block's
  square+reduce tail is short); PE consumes blocks j-interleaved.
- Y^2 = square(fp8 Y) -> bf16 split ACT/DVE/Pool ~ (11:8:1)/20 per
  block (HW-measured rates; Pool runs fp8 mults at ~0.15 eff);
  partition-reduced by bf16 matmuls (mask col -> sy2 row).
- Batched epilogue: j0's sumy/sy2/q land on partition 32, j1's on 63
  (mask at f8-image col 32/63, q via a ones-at-63 lhsT), so one
  [32:64]-partition chain — single reciprocal — serves both js.  The
  two partial w-sums ship to DRAM rows 0/31; host adds them.
- PSUM: GS{j} (DR out + warmup), Y2S{j}, big{j} (H/P/q via tag
  rotation), tns x2 = exactly 8 banks.
"""

import ml_dtypes
import numpy as np

import concourse.bacc as bacc
import concourse.bass as bass
import concourse.mybir as mybir
import concourse.tile as tile
from concourse.bass_utils import run_bass_kernel_spmd
from concourse.tile_rust import add_dep_helper

F32 = mybir.dt.float32
F32R = mybir.dt.float32r
BF16 = mybir.dt.bfloat16
F8 = mybir.dt.float8e4
ALU = mybir.AluOpType
AX = mybir.AxisListType
DR = mybir.MatmulPerfMode.DoubleRow

B, T, N, K, D = 16, 32, 6000, 32, 8
NCORES = 8
JB = B // NCORES          # batch elements per core
NCH = 47                  # ceil(6000/128) real chunks of 128 rows
NCHP = 48                 # padded chunk count (DR pairing)
PAIRS = NCHP // 2         # 24 DoubleRow pair-matmuls per j
TD = T * D                # 256
FW = 34                   # f16 image: 32 coeffs + mask + pad
FROW = NCH * FW           # 1598
KS = 64                   # f8 k-tile stride: dual-fp8 ldweights needs the
                          # outer weight step even and 16B-aligned; 64 also
                          # puts j1's sumy on out partition 63 (mask col 63)
PW = 2 * KS               # f8 pair stride
F8ROW = PAIRS * PW        # 3072
YROW = NCHP * TD          # 12288
BLOCKS = (12, 16, 20)     # chunks per Y block (ascending: first arrives fast)
NBLK = len(BLOCKS)
BOFF = (0, 12, 28)        # chunk offset of each block
# squares ACT/DVE chunks per (j, block); Pool measured ~2us/chunk on fp8
# so it carries the NS/epilogue chains instead.  Last blocks split
# asymmetrically so j0's Y2 can start while j1's squares still run.
SQ_SPLIT = {(0, 0): (7, 5), (1, 0): (7, 5),
            (0, 1): (9, 7), (1, 1): (9, 7),
            (0, 2): (11, 9), (1, 2): (11, 9)}
NWARM = 16                # PE p-state warmup matmuls
NS_ITERS = 3
EPS = 1e-8
DECAY = 0.9
PEN = 0.1

_CACHE = {}


def _build_program():
    nc = bacc.Bacc("TRN2", target_bir_lowering=False, debug=False)
    y_d = nc.declare_dram_parameter("y", [JB, 128, YROW], F8, isOutput=False)
    f_d = nc.declare_dram_parameter("f", [JB, 128, FROW], BF16, isOutput=False)
    g_d = nc.declare_dram_parameter("g", [JB, 128, F8ROW], F8, isOutput=False)
    c_d = nc.declare_dram_parameter("c32", [32, 160], F32, isOutput=False)
    w_d = nc.declare_dram_parameter("w2", [32, TD], F32, isOutput=False)
    m_d = nc.declare_dram_parameter("mk", [128, 1], BF16, isOutput=False)
    o_d = nc.declare_dram_parameter("out", [32, 2], F32, isOutput=True)

    with tile.TileContext(nc) as tc:
        with (
            tc.tile_pool(name="cpool", bufs=1) as cpool,
            tc.tile_pool(name="fpool", bufs=1) as fpool,
            tc.tile_pool(name="ypool", bufs=1) as ypool,
            tc.tile_pool(name="y2pool", bufs=1) as y2pool,
            tc.tile_pool(name="nsb", bufs=2) as nsb,
            tc.tile_pool(name="esb", bufs=2) as esb,
            tc.tile_pool(name="ps", bufs=1, space="PSUM") as ps,
        ):
            # ---- PSUM banks (8 total): GS{j}, Y2S{j}, big{j}, tns x2
            GS = [ps.tile([64, 512], F32, tag=f"GS{j}", name=f"GS{j}")
                  for j in range(JB)]
            Y2S = [ps.tile([64, 512], F32, tag=f"Y2S{j}", name=f"Y2S{j}")
                   for j in range(JB)]

            # ---- PE warmup into the GS banks (overwritten by the real
            # DoubleRow groups, which re-start the accumulation).
            wtile = cpool.tile([128, 512], BF16)
            nc.vector.memset(wtile, 0.01)
            for i in range(NWARM):
                nc.tensor.matmul(GS[i % 2][0:64, 0:512], wtile[:, 0:64],
                                 wtile, start=True, stop=True)

            # ---- DMAs: ALL stream transfers on the sync ring (a trigger
            # whose chain-wait is pending stalls its whole sequencer, so
            # the ACT/Pool engines must carry no stream triggers or their
            # squares queue behind the waits).  Global depth-2 chain in
            # arrival-shaped order: first Y blocks, then F/G, then the
            # rest of Y.
            ftile = fpool.tile([128, JB * FROW], BF16)
            gtile = fpool.tile([128, JB * F8ROW], F8)
            ytiles = {}
            for b in range(NBLK):
                for j in range(JB):
                    ytiles[(j, b)] = ypool.tile(
                        [128, BLOCKS[b] * TD], F8,
                        tag=f"yb{j}_{b}", name=f"yb{j}_{b}")
            chain = []

            def chain_dma(out, in_):
                dma = nc.sync.dma_start(out=out, in_=in_)
                if len(chain) >= 3:
                    add_dep_helper(dma.ins, chain[-3].ins, sync=True,
                                   reason="depth-3 stream chain")
                chain.append(dma)
                return dma

            def y_dma(j, b):
                chain_dma(ytiles[(j, b)][:, :],
                          y_d[j, :, BOFF[b] * TD:(BOFF[b] + BLOCKS[b]) * TD])

            y_dma(0, 0)
            y_dma(1, 0)
            chain_dma(ftile[:, 0:FROW], f_d[0, :, :])
            chain_dma(gtile[:, 0:F8ROW], g_d[0, :, :])
            chain_dma(ftile[:, FROW:2 * FROW], f_d[1, :, :])
            chain_dma(gtile[:, F8ROW:2 * F8ROW], g_d[1, :, :])
            y_dma(0, 1)
            y_dma(1, 1)
            y_dma(0, 2)
            y_dma(1, 2)

            consts = cpool.tile([32, 160], F32)
            nc.gpsimd.dma_start(out=consts, in_=c_d[:, :])
            eye = consts[:, 0:32]
            twoI = consts[:, 32:64]
            ones2d = consts[:, 64:96]
            ones32 = consts[:, 64:65]
            ones64q = consts[:, 96:160]     # zeros with col 63 = 1

            # w2 on both epilogue partitions (32 for j0, 63 for j1):
            # host ships a [32, TD] image with rows 0 and 31 = w2 so one
            # DMA lands it on partitions 32..63 (SBUF APs may only start
            # at partition 0/32/64/96)
            w2sb = cpool.tile([64, TD], F32)
            nc.gpsimd.dma_start(out=w2sb[32:64, :], in_=w_d[:, :])
            sumw = cpool.tile([64, 1], F32)
            nc.vector.reduce_sum(sumw[32:64, :], w2sb[32:64, :], axis=AX.X)

            # combined epilogue staging: cols 0:256 sumy, 256:512 sy2,
            # 512:768 q; row 32 = j0, row 63 = j1
            comb = cpool.tile([64, 3 * TD], F32)
            nc.vector.memset(comb, 0.0)

            def fch(j, c):  # chunk-c [F|mask] block [128, 33] bf16
                return ftile[:, j * FROW + c * FW: j * FROW + c * FW + 33]

            # Y2-reduce lhsT tiles: mask-only columns (col 32 for j0,
            # col 63 for j1) so sy2 lands on the epilogue partitions and
            # the Y2 matmuls have no dependency on the F image at all.
            # mk = chunk-46 row-validity mask, shipped as its own tiny
            # param so the B variants are ready before F arrives.
            mksb = cpool.tile([128, 1], BF16)
            nc.gpsimd.dma_start(out=mksb, in_=m_d[:, :])
            m0A = cpool.tile([128, 33], BF16)
            nc.gpsimd.memset(m0A, 0.0)
            nc.gpsimd.memset(m0A[:, 32:33], 1.0)
            m0B = cpool.tile([128, 33], BF16)
            nc.gpsimd.memset(m0B, 0.0)
            nc.vector.tensor_copy(m0B[:, 32:33], mksb)
            m1A = cpool.tile([128, 64], BF16)
            nc.gpsimd.memset(m1A, 0.0)
            nc.gpsimd.memset(m1A[:, 63:64], 1.0)
            m1B = cpool.tile([128, 64], BF16)
            nc.gpsimd.memset(m1B, 0.0)
            nc.vector.tensor_copy(m1B[:, 63:64], mksb)

            # ---- H = [F|mask]^T [F|mask] per j (needs only the F image)
            Hsb_j = [None, None]

            def emit_H(j):
                Hps = ps.tile([64, 512], F32, tag=f"big{j}")
                for c in range(NCH):
                    nc.tensor.matmul(Hps[0:33, 0:33], fch(j, c), fch(j, c),
                                     start=(c == 0), stop=(c == NCH - 1))
                Hsb = nsb.tile([33, 33], F32, tag="Hsb")
                nc.vector.tensor_copy(Hsb, Hps[0:33, 0:33])
                Hsb_j[j] = Hsb

            # ---- Newton-Schulz + corr-penalty step closures (PE steps
            # interleaved into the stream so the PE FIFO never head-blocks
            # on their DVE inputs).
            inv_sb = [None, None]
            quad_sb = [None, None]

            def make_steps(j):
                state = {}

                def s_trace():
                    Hsb = Hsb_j[j]
                    A = state["A"] = Hsb[0:32, 0:32]
                    state["s_row"] = Hsb[32:33, 0:32]
                    dm = nsb.tile([32, 32], F32, tag="dm")
                    nc.vector.tensor_mul(dm, A, eye)
                    dg = nsb.tile([32, 1], F32, tag="dg")
                    nc.vector.reduce_sum(dg, dm, axis=AX.X)
                    trp = ps.tile([128, 512], F32, tag="tns", bufs=2)
                    nc.tensor.matmul(trp[0:32, 0:1], ones2d, dg,
                                     start=True, stop=True)
                    rtr = nsb.tile([32, 1], F32, tag="rtr")
                    nc.vector.reciprocal(rtr, trp[0:32, 0:1])
                    c0v = nsb.tile([32, 1], F32, tag="c0v")
                    nc.vector.tensor_scalar_mul(c0v, rtr, float(K))
                    X = nsb.tile([32, 32], F32, tag="Xns", bufs=2 * NS_ITERS + 4)
                    nc.vector.tensor_scalar(X, eye, c0v, None, ALU.mult)
                    state["X"] = X
                steps = [s_trace]

                def ns_a():
                    t1 = ps.tile([128, 512], F32, tag="tns", bufs=2)
                    t1 = t1[0:32, 0:32]
                    nc.tensor.matmul(t1, state["A"], state["X"],
                                     start=True, stop=True)
                    z = nsb.tile([32, 32], F32, tag="Zns",
                                 bufs=2 * NS_ITERS + 2)
                    nc.vector.tensor_sub(z, twoI, t1)
                    state["z"] = z

                def ns_b():
                    x2 = ps.tile([128, 512], F32, tag="tns", bufs=2)
                    x2 = x2[0:32, 0:32]
                    nc.tensor.matmul(x2, state["X"], state["z"],
                                     start=True, stop=True)
                    Xn = nsb.tile([32, 32], F32, tag="Xns",
                                  bufs=2 * NS_ITERS + 4)
                    nc.vector.tensor_copy(Xn, x2)
                    state["X"] = Xn
                for _ in range(NS_ITERS):
                    steps += [ns_a, ns_b]

                def c_outer():
                    inv_sb[j] = state["X"]
                    outp = ps.tile([128, 512], F32, tag="tns", bufs=2)
                    outp = outp[0:32, 0:32]
                    nc.tensor.matmul(outp, state["s_row"], state["s_row"],
                                     start=True, stop=True)
                    covn = nsb.tile([32, 32], F32, tag="covn")
                    nc.vector.tensor_scalar_mul(covn, outp, 1.0 / N)
                    cov = nsb.tile([32, 32], F32, tag="cov")
                    nc.vector.tensor_sub(cov, state["A"], covn)
                    dm2 = nsb.tile([32, 32], F32, tag="dm2")
                    nc.vector.tensor_mul(dm2, cov, eye)
                    dg2 = nsb.tile([32, 1], F32, tag="dg2")
                    nc.vector.reduce_sum(dg2, dm2, axis=AX.X)
                    cv = nsb.tile([32, 1], F32, tag="cv")
                    nc.vector.reciprocal(cv, dg2)
                    A2 = nsb.tile([32, 32], F32, tag="A2")
                    nc.vector.tensor_mul(A2, cov, cov)
                    state["cv"] = cv
                    state["A2"] = A2

                def c_u():
                    ups = ps.tile([128, 512], F32, tag="tns", bufs=2)
                    nc.tensor.matmul(ups[0:32, 0:1], state["A2"], state["cv"],
                                     start=True, stop=True)
                    usb = nsb.tile([32, 1], F32, tag="usb")
                    nc.vector.tensor_copy(usb, ups[0:32, 0:1])
                    state["usb"] = usb

                def c_q():
                    qd = ps.tile([128, 512], F32, tag="tns", bufs=2)
                    nc.tensor.matmul(qd[32:33, 0:1], state["usb"], state["cv"],
                                     start=True, stop=True)
                    qsb = nsb.tile([33, 1], F32, tag="qsb")
                    nc.vector.tensor_copy(qsb[32:33, :], qd[32:33, 0:1])
                    quad_sb[j] = qsb
                steps += [c_outer, c_u, c_q]
                return steps

            pending = {0: make_steps(0), 1: make_steps(1)}

            y2tiles = {}

            def emit_squares(j, b):
                yt = ytiles[(j, b)]
                y2t = y2pool.tile([128, BLOCKS[b] * TD], BF16,
                                  tag=f"y2_{j}_{b}", name=f"y2_{j}_{b}")
                na, nd = SQ_SPLIT[(j, b)]
                a = na * TD
                h = a + (nd // 2) * TD
                nc.scalar.square(y2t[:, 0:a], yt[:, 0:a])
                nc.vector.tensor_mul(y2t[:, a:h], yt[:, a:h], yt[:, a:h])
                nc.vector.tensor_mul(y2t[:, h:], yt[:, h:], yt[:, h:])
                y2tiles[(j, b)] = y2t

            def emit_dr(j, b, pop_steps=False):
                yt = ytiles[(j, b)]
                steps = pending[j]
                for p in range(BLOCKS[b] // 2):
                    P = BOFF[b] // 2 + p
                    lhsT = gtile[:, j * F8ROW + P * PW: j * F8ROW + (P + 1) * PW
                                 ].rearrange("p (two m) -> p two m", two=2)
                    rhs = yt[:, p * 2 * TD:(p + 1) * 2 * TD
                             ].rearrange("p (two f) -> p two f", two=2)
                    nc.tensor.matmul(GS[j][0:64, 0:256], lhsT, rhs,
                                     start=(P == 0), stop=(P == PAIRS - 1),
                                     perf_mode=DR)
                    if pop_steps and p % 2 == 1 and steps:
                        steps.pop(0)()

            def emit_y2mm(j, b, pop_steps=False):
                y2t = y2tiles[(j, b)]
                steps = pending[j]
                for lc in range(BLOCKS[b]):
                    c = BOFF[b] + lc
                    if c >= NCH:
                        continue
                    if j == 0:
                        lhsT = m0B if c == NCH - 1 else m0A
                        out = Y2S[0][0:33, 0:256]
                    else:
                        lhsT = m1B if c == NCH - 1 else m1A
                        out = Y2S[1][0:64, 0:256]
                    nc.tensor.matmul(out, lhsT,
                                     y2t[:, lc * TD:(lc + 1) * TD],
                                     start=(c == 0), stop=(c == NCH - 1))
                    if pop_steps and lc % 2 == 1 and steps:
                        steps.pop(0)()

            qps_j = [None, None]

            def phase1(j):
                """sy2-independent epilogue: P = inv M, W, q.  Runs right
                after DR(j) stops; comb staging happens in phase2."""
                Gsb = esb.tile([32, TD], F32, tag="Gsb")
                nc.vector.tensor_copy(Gsb, GS[j][0:32, 0:256])
                Pps = ps.tile([64, 512], F32, tag=f"big{j}")
                nc.tensor.matmul(Pps[0:32, 0:TD], inv_sb[j], Gsb,
                                 start=True, stop=True)
                W = esb.tile([32, TD], F32, tag="W")
                nc.vector.tensor_mul(W, Gsb, Pps[0:32, 0:TD])
                qps = ps.tile([64, 512], F32, tag=f"big{j}")
                if j == 0:
                    nc.tensor.matmul(qps[32:33, 0:TD], ones32, W,
                                     start=True, stop=True)
                else:
                    nc.tensor.matmul(qps[0:64, 0:TD], ones64q, W,
                                     start=True, stop=True)
                qps_j[j] = qps

            def phase2():
                """Batched tail for both js on partitions 32..63.  SBUF
                APs may only start at partitions 0/32/64/96, so j1's row
                63 is staged via [32:64] block copies (its rows 32..62
                are zeros/junk), then j0's row 32 overwrites."""
                R = slice(32, 64)
                nc.vector.tensor_copy(comb[R, 0:TD], GS[1][32:64, 0:256])
                nc.vector.tensor_copy(comb[32:33, 0:TD], GS[0][32:33, 0:256])
                nc.vector.tensor_copy(comb[R, TD:2 * TD],
                                      Y2S[1][32:64, 0:256])
                nc.vector.tensor_copy(comb[32:33, TD:2 * TD],
                                      Y2S[0][32:33, 0:256])
                nc.vector.tensor_copy(comb[R, 2 * TD:3 * TD],
                                      qps_j[1][32:64, 0:TD])
                nc.vector.tensor_copy(comb[32:33, 2 * TD:3 * TD],
                                      qps_j[0][32:33, 0:TD])
                sumy = comb[R, 0:TD]
                sy2 = comb[R, TD:2 * TD]
                qrow = comb[R, 2 * TD:3 * TD]
                sstot_a = esb.tile([64, TD], F32, tag="sstot_a")
                nc.vector.scalar_tensor_tensor(
                    sstot_a[R, :], sumy, -1.0 / N, sumy, ALU.mult, ALU.mult)
                sstot = esb.tile([64, TD], F32, tag="sstot")
                nc.vector.scalar_tensor_tensor(
                    sstot[R, :], sstot_a[R, :], EPS, sy2, ALU.add, ALU.add)
                rec = esb.tile([64, TD], F32, tag="rec")
                nc.vector.reciprocal(rec[R, :], sstot[R, :])
                wrec = esb.tile([64, TD], F32, tag="wrec")
                nc.vector.tensor_mul(wrec[R, :], rec[R, :], w2sb[R, :])
                tA = esb.tile([64, TD], F32, tag="tA")
                accA = esb.tile([64, 1], F32, tag="accA")
                nc.vector.scalar_tensor_tensor(
                    tA[R, :], sy2, 1.0, wrec[R, :],
                    ALU.mult, ALU.mult, accum_out=accA[R, :])
                tB = esb.tile([64, TD], F32, tag="tB")
                accB = esb.tile([64, 1], F32, tag="accB")
                nc.vector.scalar_tensor_tensor(
                    tB[R, :], qrow, 1.0, wrec[R, :],
                    ALU.mult, ALU.mult, accum_out=accB[R, :])
                d1 = esb.tile([64, 1], F32, tag="d1")
                nc.vector.tensor_sub(d1[R, :], sumw[R, :], accA[R, :])
                outsb = cpool.tile([64, 2], F32)
                nc.vector.memset(outsb, 0.0)
                nc.vector.tensor_add(outsb[R, 0:1], d1[R, :], accB[R, :])
                nc.vector.tensor_add(outsb[32:33, 1:2],
                                     quad_sb[0][32:33, :],
                                     quad_sb[1][32:33, :])
                nc.sync.dma_start(out=o_d[:, :], in_=outsb[R, 0:2])

            # ---- stream emission in DMA-arrival order (PE is in-order,
            # so a stalled instruction blocks everything behind it):
            # y(b0) -> Y2(b0);  f16 -> H;  g -> DR;  interleave NS.
            emit_squares(0, 0)
            emit_squares(1, 0)
            emit_y2mm(0, 0)
            emit_y2mm(1, 0)
            emit_H(0)
            emit_dr(0, 0, pop_steps=True)
            emit_H(1)
            emit_dr(1, 0, pop_steps=True)
            emit_squares(0, 1)
            emit_squares(1, 1)
            emit_y2mm(0, 1, pop_steps=True)
            emit_y2mm(1, 1, pop_steps=True)
            emit_dr(0, 1, pop_steps=True)
            emit_dr(1, 1, pop_steps=True)
            emit_squares(0, 2)
            emit_dr(0, 2)
            phase1(0)
            emit_squares(1, 2)
            emit_dr(1, 2)
            emit_y2mm(0, 2)
            phase1(1)
            emit_y2mm(1, 2)
            phase2()

    nc.compile()
    return nc


def _prepare_in_maps(preds, y_ts, importance):
    preds = np.ascontiguousarray(preds, dtype=np.float32)
    y_ts = np.ascontiguousarray(y_ts, dtype=np.float32)
    importance = np.ascontiguousarray(importance, dtype=np.float32)

    bf16 = ml_dtypes.bfloat16
    f8 = ml_dtypes.float8_e4m3fn
    NPAD = NCHP * 128     # 6144

    # Y image: yimg[b, p, c*TD + t*D + d] = fp8(y_ts[b, t, c*128+p, d])
    ypad = np.zeros((B, T, NPAD, D), dtype=f8)
    ypad[:, :, :N, :] = y_ts.astype(f8)
    yimg = np.ascontiguousarray(
        ypad.reshape(B, T, NCHP, 128, D).transpose(0, 3, 2, 1, 4)
    ).reshape(B, 128, YROW)

    # F bf16 image: fimg[b, p, c*FW + k]; col 32 = valid-mask
    fpad = np.zeros((B, NCH * 128, FW), dtype=bf16)
    fpad[:, :N, :K] = preds.astype(bf16)
    fpad[:, :N, K] = 1.0
    fimg = np.ascontiguousarray(
        fpad.reshape(B, NCH, 128, FW).transpose(0, 2, 1, 3)
    ).reshape(B, 128, FROW)

    # F fp8 image, 48 chunks, pair-major for DoubleRow lhsT; k-tile
    # stride KS=64; mask col 32 for even batch (j0), 63 for odd (j1)
    gpad = np.zeros((B, NPAD, KS), dtype=f8)
    gpad[:, :N, :K] = preds.astype(f8)
    gpad[0::2, :N, 32] = 1.0
    gpad[1::2, :N, 63] = 1.0
    gimg = np.ascontiguousarray(
        gpad.reshape(B, NCHP, 128, KS).transpose(0, 2, 1, 3)
    ).reshape(B, 128, F8ROW)

    c32 = np.zeros((32, 160), dtype=np.float32)
    c32[:, 0:32] = np.eye(32, dtype=np.float32)
    c32[:, 32:64] = 2.0 * np.eye(32, dtype=np.float32)
    c32[:, 64:96] = 1.0
    c32[:, 96 + 63] = 1.0

    decay = DECAY ** np.arange(T, dtype=np.float32)
    w2row = (decay[:, None] * importance[None, :].astype(np.float32)
             ).reshape(TD)
    w2 = np.zeros((32, TD), dtype=np.float32)
    w2[0] = w2row
    w2[31] = w2row

    mk = np.zeros((128, 1), dtype=bf16)
    mk[:N - (NCH - 1) * 128, 0] = 1.0

    in_maps = []
    for i in range(NCORES):
        in_maps.append({
            "y": np.ascontiguousarray(yimg[i * JB:(i + 1) * JB]),
            "f": np.ascontiguousarray(fimg[i * JB:(i + 1) * JB]),
            "g": np.ascontiguousarray(gimg[i * JB:(i + 1) * JB]),
            "c32": c32,
            "w2": w2,
            "mk": mk,
        })
    return in_maps


def _combine(results):
    loss = 0.0
    for r in results:
        w_total = float(r["out"][0, 0]) + float(r["out"][31, 0])
        q_total = float(r["out"][0, 1])
        loss += (-w_total / T + PEN * (q_total - JB * K)) / B
    return np.float32(loss)


def run_on_device(preds, y_ts, importance, trace=False, **spmd_kwargs):
    if "nc" not in _CACHE:
        _CACHE["nc"] = _build_program()
    nc = _CACHE["nc"]
    in_maps = _prepare_in_maps(preds, y_ts, importance)
    res = run_bass_kernel_spmd(
        nc, in_maps, list(range(NCORES)), trace=trace, **spmd_kwargs
    )
    return _combine(res.results), res


def kernel(preds, y_ts, importance):
    loss, _ = run_on_device(preds, y_ts, importance, trace=False)
    return loss


# revision 17
# speedup vs baseline: 1.1024x; 1.0362x over previous
"""Trainium2 Bass kernel for AccumulativeGainLoss — fp8-stream version.

Data-parallel over B across 8 NeuronCores (JB=2 batch elements per core).

Math (same restructure as the bf16 baseline, validated on host):
    H    = [F|1]^T [F|1]      bf16 PE, PSUM accum         [33,33]
    inv  = (F^T F)^{-1}       Newton-Schulz 3 iters
    M;sumy = [F|m]^T Y        fp8 DoubleRow PE stream     [64,256]
    sy2  = mask^T Y^2         bf16 PE reduce of squares
    q    = colsum(M * inv M);  ss_res = sy2 - q
    ss_tot = sy2 - sumy^2/N + EPS;  r2 = 1 - ss_res/ss_tot
    wsum = sum(w * r2);  cov = FtF - s s^T/N; quad = c^T (cov*cov) c
loss = mean_b(-wsum/T) + 0.1 * mean_b(quad - K)

Design (v4):
- Y ships as fp8e4m3 (3.1MB/core vs 6.2 bf16); M/sumy stream runs as
  DoubleRow fp8 matmuls (K=256 per matmul, 256 cyc on HW).  Host sim:
  rel err ~7.6e-4 (Y^2 must stay bf16; fp8 Y^2 costs 3e-3).
- Two parallel HWDGE DMA rings (sync=j0, scalar=j1), each depth-2
  chained, Y in 20/16/12-chunk blocks (descending so the last block's
  square+reduce tail is short); PE consumes blocks j-interleaved.
- Y^2 = square(fp8 Y) -> bf16 split ACT/DVE/Pool ~ (11:8:1)/20 per
  block (HW-measured rates; Pool runs fp8 mults at ~0.15 eff);
  partition-reduced by bf16 matmuls (mask col -> sy2 row).
- Batched epilogue: j0's sumy/sy2/q land on partition 32, j1's on 63
  (mask at f8-image col 32/63, q via a ones-at-63 lhsT), so one
  [32:64]-partition chain — single reciprocal — serves both js.  The
  two partial w-sums ship to DRAM rows 0/31; host adds them.
- PSUM: GS{j} (DR out + warmup), Y2S{j}, big{j} (H/P/q via tag
  rotation), tns x2 = exactly 8 banks.
"""

import ml_dtypes
import numpy as np

import concourse.bacc as bacc
import concourse.bass as bass
import concourse.mybir as mybir
import concourse.tile as tile
from concourse.bass_utils import run_bass_kernel_spmd
from concourse.tile_rust import add_dep_helper

F32 = mybir.dt.float32
F32R = mybir.dt.float32r
BF16 = mybir.dt.bfloat16
F8 = mybir.dt.float8e4
ALU = mybir.AluOpType
AX = mybir.AxisListType
DR = mybir.MatmulPerfMode.DoubleRow

B, T, N, K, D = 16, 32, 6000, 32, 8
NCORES = 8
JB = B // NCORES          # batch elements per core
NCH = 47                  # ceil(6000/128) real chunks of 128 rows
NCHP = 48                 # padded chunk count (DR pairing)
PAIRS = NCHP // 2         # 24 DoubleRow pair-matmuls per j
TD = T * D                # 256
FW = 34                   # f16 image: 32 coeffs + mask + pad
FROW = NCH * FW           # 1598
KS = 64                   # f8 k-tile stride: dual-fp8 ldweights needs the
                          # outer weight step even and 16B-aligned; 64 also
                          # puts j1's sumy on out partition 63 (mask col 63)
PW = 2 * KS               # f8 pair stride
F8ROW = PAIRS * PW        # 3072
YROW = NCHP * TD          # 12288
BLOCKS = (12, 16, 20)     # chunks per Y block (ascending: first arrives fast)
NBLK = len(BLOCKS)
BOFF = (0, 12, 28)        # chunk offset of each block
# squares ACT/DVE chunks per (j, block); Pool measured ~2us/chunk on fp8
# so it carries the NS/epilogue chains instead.  Last blocks split
# asymmetrically so j0's Y2 can start while j1's squares still run.
SQ_SPLIT = {(0, 0): (8, 4), (1, 0): (8, 4),
            (0, 1): (10, 6), (1, 1): (10, 6),
            (0, 2): (13, 7), (1, 2): (13, 7)}
NWARM = 12                # PE p-state warmup matmuls
NS_ITERS = 3
EPS = 1e-8
DECAY = 0.9
PEN = 0.1

_CACHE = {}


def _build_program():
    nc = bacc.Bacc("TRN2", target_bir_lowering=False, debug=False)
    y_d = nc.declare_dram_parameter("y", [JB, 128, YROW], F8, isOutput=False)
    f_d = nc.declare_dram_parameter("f", [JB, 128, FROW], BF16, isOutput=False)
    g_d = nc.declare_dram_parameter("g", [JB, 128, F8ROW], F8, isOutput=False)
    c_d = nc.declare_dram_parameter("c32", [32, 160], F32, isOutput=False)
    w_d = nc.declare_dram_parameter("w2", [32, TD], F32, isOutput=False)
    m_d = nc.declare_dram_parameter("mk", [128, 1], BF16, isOutput=False)
    o_d = nc.declare_dram_parameter("out", [32, 4], F32, isOutput=True)

    with tile.TileContext(nc) as tc:
        with (
            tc.tile_pool(name="cpool", bufs=1) as cpool,
            tc.tile_pool(name="fpool", bufs=1) as fpool,
            tc.tile_pool(name="ypool", bufs=1) as ypool,
            tc.tile_pool(name="y2pool", bufs=1) as y2pool,
            tc.tile_pool(name="nsb", bufs=2) as nsb,
            tc.tile_pool(name="esb", bufs=2) as esb,
            tc.tile_pool(name="ps", bufs=1, space="PSUM") as ps,
        ):
            # ---- PSUM banks (8 total): GS{j}, Y2S{j}, big{j}, tns x2
            GS = [ps.tile([64, 512], F32, tag=f"GS{j}", name=f"GS{j}")
                  for j in range(JB)]
            Y2S = [ps.tile([64, 512], F32, tag=f"Y2S{j}", name=f"Y2S{j}")
                   for j in range(JB)]

            # ---- PE warmup into the GS banks (overwritten by the real
            # DoubleRow groups, which re-start the accumulation).
            wtile = cpool.tile([128, 512], BF16)
            nc.vector.memset(wtile, 0.01)
            for i in range(NWARM):
                nc.tensor.matmul(GS[i % 2][0:64, 0:512], wtile[:, 0:64],
                                 wtile, start=True, stop=True)

            # ---- DMAs: ALL stream transfers on the sync ring (a trigger
            # whose chain-wait is pending stalls its whole sequencer, so
            # the ACT/Pool engines must carry no stream triggers or their
            # squares queue behind the waits).  Global depth-2 chain in
            # arrival-shaped order: first Y blocks, then F/G, then the
            # rest of Y.
            ftile = fpool.tile([128, JB * FROW], BF16)
            gtile = fpool.tile([128, JB * F8ROW], F8)
            ytiles = {}
            for b in range(NBLK):
                for j in range(JB):
                    ytiles[(j, b)] = ypool.tile(
                        [128, BLOCKS[b] * TD], F8,
                        tag=f"yb{j}_{b}", name=f"yb{j}_{b}")
            chain = []

            def chain_dma(out, in_):
                dma = nc.sync.dma_start(out=out, in_=in_)
                if len(chain) >= 3:
                    add_dep_helper(dma.ins, chain[-3].ins, sync=True,
                                   reason="depth-3 stream chain")
                chain.append(dma)
                return dma

            def y_dma(j, b):
                chain_dma(ytiles[(j, b)][:, :],
                          y_d[j, :, BOFF[b] * TD:(BOFF[b] + BLOCKS[b]) * TD])

            y_dma(0, 0)
            y_dma(1, 0)
            chain_dma(ftile[:, 0:FROW], f_d[0, :, :])
            chain_dma(gtile[:, 0:F8ROW], g_d[0, :, :])
            chain_dma(ftile[:, FROW:2 * FROW], f_d[1, :, :])
            chain_dma(gtile[:, F8ROW:2 * F8ROW], g_d[1, :, :])
            y_dma(0, 1)
            y_dma(1, 1)
            y_dma(0, 2)
            y_dma(1, 2)

            consts = cpool.tile([32, 160], F32)
            nc.gpsimd.dma_start(out=consts, in_=c_d[:, :])
            eye = consts[:, 0:32]
            twoI = consts[:, 32:64]
            ones2d = consts[:, 64:96]
            ones32 = consts[:, 64:65]
            ones64q = consts[:, 96:160]     # zeros with col 63 = 1

            # w2 on both epilogue partitions (32 for j0, 63 for j1):
            # host ships a [32, TD] image with rows 0 and 31 = w2 so one
            # DMA lands it on partitions 32..63 (SBUF APs may only start
            # at partition 0/32/64/96)
            w2sb = cpool.tile([64, TD], F32)
            nc.gpsimd.dma_start(out=w2sb[32:64, :], in_=w_d[:, :])
            sumw = cpool.tile([64, 1], F32)
            nc.vector.reduce_sum(sumw[32:64, :], w2sb[32:64, :], axis=AX.X)

            outsb = cpool.tile([64, 4], F32)
            nc.vector.memset(outsb, 0.0)

            def fch(j, c):  # chunk-c [F|mask] block [128, 33] bf16
                return ftile[:, j * FROW + c * FW: j * FROW + c * FW + 33]

            # Y2-reduce lhsT tiles: mask-only columns (col 32 for j0,
            # col 63 for j1) so sy2 lands on the epilogue partitions and
            # the Y2 matmuls have no dependency on the F image at all.
            # mk = chunk-46 row-validity mask, shipped as its own tiny
            # param so the B variants are ready before F arrives.
            mksb = cpool.tile([128, 1], BF16)
            nc.gpsimd.dma_start(out=mksb, in_=m_d[:, :])
            m0A = cpool.tile([128, 33], BF16)
            nc.gpsimd.memset(m0A, 0.0)
            nc.gpsimd.memset(m0A[:, 32:33], 1.0)
            m0B = cpool.tile([128, 33], BF16)
            nc.gpsimd.memset(m0B, 0.0)
            nc.vector.tensor_copy(m0B[:, 32:33], mksb)
            m1A = cpool.tile([128, 64], BF16)
            nc.gpsimd.memset(m1A, 0.0)
            nc.gpsimd.memset(m1A[:, 63:64], 1.0)
            m1B = cpool.tile([128, 64], BF16)
            nc.gpsimd.memset(m1B, 0.0)
            nc.vector.tensor_copy(m1B[:, 63:64], mksb)

            # ---- H = [F|mask]^T [F|mask] per j (needs only the F image)
            Hsb_j = [None, None]

            def emit_H(j):
                Hps = ps.tile([64, 512], F32, tag=f"big{j}")
                for c in range(NCH):
                    nc.tensor.matmul(Hps[0:33, 0:33], fch(j, c), fch(j, c),
                                     start=(c == 0), stop=(c == NCH - 1))
                Hsb = nsb.tile([33, 33], F32, tag="Hsb")
                nc.vector.tensor_copy(Hsb, Hps[0:33, 0:33])
                Hsb_j[j] = Hsb

            # ---- Newton-Schulz + corr-penalty step closures (PE steps
            # interleaved into the stream so the PE FIFO never head-blocks
            # on their DVE inputs).
            inv_sb = [None, None]
            quad_sb = [None, None]

            def make_steps(j):
                state = {}

                def s_trace():
                    Hsb = Hsb_j[j]
                    A = state["A"] = Hsb[0:32, 0:32]
                    state["s_row"] = Hsb[32:33, 0:32]
                    dm = nsb.tile([32, 32], F32, tag="dm")
                    nc.vector.tensor_mul(dm, A, eye)
                    dg = nsb.tile([32, 1], F32, tag="dg")
                    nc.vector.reduce_sum(dg, dm, axis=AX.X)
                    trp = ps.tile([128, 512], F32, tag="tns", bufs=2)
                    nc.tensor.matmul(trp[0:32, 0:1], ones2d, dg,
                                     start=True, stop=True)
                    rtr = nsb.tile([32, 1], F32, tag="rtr")
                    nc.vector.reciprocal(rtr, trp[0:32, 0:1])
                    c0v = nsb.tile([32, 1], F32, tag="c0v")
                    nc.vector.tensor_scalar_mul(c0v, rtr, float(K))
                    X = nsb.tile([32, 32], F32, tag="Xns", bufs=2 * NS_ITERS + 4)
                    nc.vector.tensor_scalar(X, eye, c0v, None, ALU.mult)
                    state["X"] = X
                steps = [s_trace]

                def ns_a():
                    t1 = ps.tile([128, 512], F32, tag="tns", bufs=2)
                    t1 = t1[0:32, 0:32]
                    nc.tensor.matmul(t1, state["A"], state["X"],
                                     start=True, stop=True)
                    z = nsb.tile([32, 32], F32, tag="Zns",
                                 bufs=2 * NS_ITERS + 2)
                    nc.vector.tensor_sub(z, twoI, t1)
                    state["z"] = z

                def ns_b():
                    x2 = ps.tile([128, 512], F32, tag="tns", bufs=2)
                    x2 = x2[0:32, 0:32]
                    nc.tensor.matmul(x2, state["X"], state["z"],
                                     start=True, stop=True)
                    Xn = nsb.tile([32, 32], F32, tag="Xns",
                                  bufs=2 * NS_ITERS + 4)
                    nc.vector.tensor_copy(Xn, x2)
                    state["X"] = Xn
                for _ in range(NS_ITERS):
                    steps += [ns_a, ns_b]

                def c_outer():
                    inv_sb[j] = state["X"]
                    outp = ps.tile([128, 512], F32, tag="tns", bufs=2)
                    outp = outp[0:32, 0:32]
                    nc.tensor.matmul(outp, state["s_row"], state["s_row"],
                                     start=True, stop=True)
                    covn = nsb.tile([32, 32], F32, tag="covn")
                    nc.vector.tensor_scalar_mul(covn, outp, 1.0 / N)
                    cov = nsb.tile([32, 32], F32, tag="cov")
                    nc.vector.tensor_sub(cov, state["A"], covn)
                    dm2 = nsb.tile([32, 32], F32, tag="dm2")
                    nc.vector.tensor_mul(dm2, cov, eye)
                    dg2 = nsb.tile([32, 1], F32, tag="dg2")
                    nc.vector.reduce_sum(dg2, dm2, axis=AX.X)
                    cv = nsb.tile([32, 1], F32, tag="cv")
                    nc.vector.reciprocal(cv, dg2)
                    A2 = nsb.tile([32, 32], F32, tag="A2")
                    nc.vector.tensor_mul(A2, cov, cov)
                    state["cv"] = cv
                    state["A2"] = A2

                def c_u():
                    ups = ps.tile([128, 512], F32, tag="tns", bufs=2)
                    nc.tensor.matmul(ups[0:32, 0:1], state["A2"], state["cv"],
                                     start=True, stop=True)
                    usb = nsb.tile([32, 1], F32, tag="usb")
                    nc.vector.tensor_copy(usb, ups[0:32, 0:1])
                    state["usb"] = usb

                def c_q():
                    qd = ps.tile([128, 512], F32, tag="tns", bufs=2)
                    nc.tensor.matmul(qd[32:33, 0:1], state["usb"], state["cv"],
                                     start=True, stop=True)
                    qsb = nsb.tile([33, 1], F32, tag="qsb")
                    nc.vector.tensor_copy(qsb[32:33, :], qd[32:33, 0:1])
                    quad_sb[j] = qsb
                steps += [c_outer, c_u, c_q]
                return steps

            pending = {0: make_steps(0), 1: make_steps(1)}

            y2tiles = {}

            def emit_squares(j, b):
                yt = ytiles[(j, b)]
                y2t = y2pool.tile([128, BLOCKS[b] * TD], BF16,
                                  tag=f"y2_{j}_{b}", name=f"y2_{j}_{b}")
                na, nd = SQ_SPLIT[(j, b)]
                a = na * TD
                h = a + (nd // 2) * TD
                nc.scalar.square(y2t[:, 0:a], yt[:, 0:a])
                nc.vector.tensor_mul(y2t[:, a:h], yt[:, a:h], yt[:, a:h])
                nc.vector.tensor_mul(y2t[:, h:], yt[:, h:], yt[:, h:])
                y2tiles[(j, b)] = y2t

            def emit_dr(j, b, pop_steps=False):
                yt = ytiles[(j, b)]
                steps = pending[j]
                for p in range(BLOCKS[b] // 2):
                    P = BOFF[b] // 2 + p
                    lhsT = gtile[:, j * F8ROW + P * PW: j * F8ROW + (P + 1) * PW
                                 ].rearrange("p (two m) -> p two m", two=2)
                    rhs = yt[:, p * 2 * TD:(p + 1) * 2 * TD
                             ].rearrange("p (two f) -> p two f", two=2)
                    nc.tensor.matmul(GS[j][0:64, 0:256], lhsT, rhs,
                                     start=(P == 0), stop=(P == PAIRS - 1),
                                     perf_mode=DR)
                    if pop_steps and p % 2 == 1 and steps:
                        steps.pop(0)()

            def emit_y2mm(j, b, pop_steps=False):
                y2t = y2tiles[(j, b)]
                steps = pending[j]
                for lc in range(BLOCKS[b]):
                    c = BOFF[b] + lc
                    if c >= NCH:
                        continue
                    if j == 0:
                        lhsT = m0B if c == NCH - 1 else m0A
                        out = Y2S[0][0:33, 0:256]
                    else:
                        lhsT = m1B if c == NCH - 1 else m1A
                        out = Y2S[1][0:64, 0:256]
                    nc.tensor.matmul(out, lhsT,
                                     y2t[:, lc * TD:(lc + 1) * TD],
                                     start=(c == 0), stop=(c == NCH - 1))
                    if pop_steps and lc % 2 == 1 and steps:
                        steps.pop(0)()

            qps_j = [None, None]

            def phase1(j):
                """sy2-independent epilogue: P = inv M, W, q.  Runs right
                after DR(j) stops; comb staging happens in phase2."""
                Gsb = esb.tile([32, TD], F32, tag="Gsb")
                nc.vector.tensor_copy(Gsb, GS[j][0:32, 0:256])
                Pps = ps.tile([64, 512], F32, tag=f"big{j}")
                nc.tensor.matmul(Pps[0:32, 0:TD], inv_sb[j], Gsb,
                                 start=True, stop=True)
                W = esb.tile([32, TD], F32, tag="W")
                nc.vector.tensor_mul(W, Gsb, Pps[0:32, 0:TD])
                qps = ps.tile([64, 512], F32, tag=f"big{j}")
                if j == 0:
                    nc.tensor.matmul(qps[32:33, 0:TD], ones32, W,
                                     start=True, stop=True)
                else:
                    nc.tensor.matmul(qps[0:64, 0:TD], ones64q, W,
                                     start=True, stop=True)
                qps_j[j] = qps

            sa_j = [None, None]

            def chain_sa(j):
                # sstot_a = -sumy^2/N, straight from the GS PSUM row(s);
                # needs only the DR group stop.
                R = slice(32, 33) if j == 0 else slice(32, 64)
                s_sb = esb.tile([64, TD], F32, tag=f"ssb{j}")
                nc.vector.tensor_copy(s_sb[R, :], GS[j][R, 0:256])
                sa = esb.tile([64, TD], F32, tag=f"sa{j}")
                nc.vector.scalar_tensor_tensor(
                    sa[R, :], s_sb[R, :], -1.0 / N, s_sb[R, :],
                    ALU.mult, ALU.mult)
                sa_j[j] = sa

            def chain_rest(j):
                # rest of the w-sum chain; needs Y2S[j] (sy2) complete.
                # j0 runs mid-stream, j1 is the tail.  Ops read the PSUM
                # rows directly (no SBUF staging); j1 spans rows 32:64
                # (only row 63 real) since SBUF APs can't start at 63.
                R = slice(32, 33) if j == 0 else slice(32, 64)
                sy2P = Y2S[j][R, 0:256]
                qP = qps_j[j][R, 0:TD]
                sstot = esb.tile([64, TD], F32, tag=f"st{j}")
                nc.vector.scalar_tensor_tensor(
                    sstot[R, :], sa_j[j][R, :], EPS, sy2P, ALU.add, ALU.add)
                rec = esb.tile([64, TD], F32, tag=f"rec{j}")
                nc.vector.reciprocal(rec[R, :], sstot[R, :])
                wrec = esb.tile([64, TD], F32, tag=f"wrec{j}")
                nc.vector.tensor_mul(wrec[R, :], rec[R, :], w2sb[R, :])
                tA = esb.tile([64, TD], F32, tag=f"tA{j}")
                accA = esb.tile([64, 1], F32, tag=f"accA{j}")
                nc.vector.scalar_tensor_tensor(
                    tA[R, :], sy2P, 1.0, wrec[R, :],
                    ALU.mult, ALU.mult, accum_out=accA[R, :])
                tB = esb.tile([64, TD], F32, tag=f"tB{j}")
                accB = esb.tile([64, 1], F32, tag=f"accB{j}")
                nc.vector.scalar_tensor_tensor(
                    tB[R, :], qP, 1.0, wrec[R, :],
                    ALU.mult, ALU.mult, accum_out=accB[R, :])
                d1 = esb.tile([64, 1], F32, tag=f"d1{j}")
                nc.vector.tensor_sub(d1[R, :], sumw[R, :], accA[R, :])
                nc.vector.tensor_add(outsb[R, j:j + 1], d1[R, :], accB[R, :])

            def finish():
                nc.vector.tensor_add(outsb[32:33, 2:3],
                                     quad_sb[0][32:33, :],
                                     quad_sb[1][32:33, :])
                nc.sync.dma_start(out=o_d[:, :], in_=outsb[32:64, 0:4])

            # ---- stream emission in DMA-arrival order (PE is in-order,
            # so a stalled instruction blocks everything behind it):
            # y(b0) -> Y2(b0);  f16 -> H;  g -> DR;  interleave NS.
            emit_squares(0, 0)
            emit_squares(1, 0)
            emit_y2mm(0, 0)
            emit_y2mm(1, 0)
            emit_H(0)
            emit_dr(0, 0, pop_steps=True)
            emit_H(1)
            emit_dr(1, 0, pop_steps=True)
            emit_squares(0, 1)
            emit_squares(1, 1)
            emit_y2mm(0, 1, pop_steps=True)
            emit_y2mm(1, 1, pop_steps=True)
            emit_dr(0, 1, pop_steps=True)
            emit_dr(1, 1, pop_steps=True)
            emit_squares(0, 2)
            emit_dr(0, 2)
            phase1(0)
            chain_sa(0)
            emit_squares(1, 2)
            emit_dr(1, 2)
            phase1(1)
            chain_sa(1)
            emit_y2mm(0, 2)
            chain_rest(0)
            emit_y2mm(1, 2)
            chain_rest(1)
            finish()

    nc.compile()
    return nc


def _prepare_in_maps(preds, y_ts, importance):
    preds = np.ascontiguousarray(preds, dtype=np.float32)
    y_ts = np.ascontiguousarray(y_ts, dtype=np.float32)
    importance = np.ascontiguousarray(importance, dtype=np.float32)

    bf16 = ml_dtypes.bfloat16
    f8 = ml_dtypes.float8_e4m3fn
    NPAD = NCHP * 128     # 6144

    # Y image: yimg[b, p, c*TD + t*D + d] = fp8(y_ts[b, t, c*128+p, d])
    ypad = np.zeros((B, T, NPAD, D), dtype=f8)
    ypad[:, :, :N, :] = y_ts.astype(f8)
    yimg = np.ascontiguousarray(
        ypad.reshape(B, T, NCHP, 128, D).transpose(0, 3, 2, 1, 4)
    ).reshape(B, 128, YROW)

    # F bf16 image: fimg[b, p, c*FW + k]; col 32 = valid-mask
    fpad = np.zeros((B, NCH * 128, FW), dtype=bf16)
    fpad[:, :N, :K] = preds.astype(bf16)
    fpad[:, :N, K] = 1.0
    fimg = np.ascontiguousarray(
        fpad.reshape(B, NCH, 128, FW).transpose(0, 2, 1, 3)
    ).reshape(B, 128, FROW)

    # F fp8 image, 48 chunks, pair-major for DoubleRow lhsT; k-tile
    # stride KS=64; mask col 32 for even batch (j0), 63 for odd (j1)
    gpad = np.zeros((B, NPAD, KS), dtype=f8)
    gpad[:, :N, :K] = preds.astype(f8)
    gpad[0::2, :N, 32] = 1.0
    gpad[1::2, :N, 63] = 1.0
    gimg = np.ascontiguousarray(
        gpad.reshape(B, NCHP, 128, KS).transpose(0, 2, 1, 3)
    ).reshape(B, 128, F8ROW)

    c32 = np.zeros((32, 160), dtype=np.float32)
    c32[:, 0:32] = np.eye(32, dtype=np.float32)
    c32[:, 32:64] = 2.0 * np.eye(32, dtype=np.float32)
    c32[:, 64:96] = 1.0
    c32[:, 96 + 63] = 1.0

    decay = DECAY ** np.arange(T, dtype=np.float32)
    w2row = (decay[:, None] * importance[None, :].astype(np.float32)
             ).reshape(TD)
    w2 = np.zeros((32, TD), dtype=np.float32)
    w2[0] = w2row
    w2[31] = w2row

    mk = np.zeros((128, 1), dtype=bf16)
    mk[:N - (NCH - 1) * 128, 0] = 1.0

    in_maps = []
    for i in range(NCORES):
        in_maps.append({
            "y": np.ascontiguousarray(yimg[i * JB:(i + 1) * JB]),
            "f": np.ascontiguousarray(fimg[i * JB:(i + 1) * JB]),
            "g": np.ascontiguousarray(gimg[i * JB:(i + 1) * JB]),
            "c32": c32,
            "w2": w2,
            "mk": mk,
        })
    return in_maps


def _combine(results):
    loss = 0.0
    for r in results:
        w_total = float(r["out"][0, 0]) + float(r["out"][31, 1])
        q_total = float(r["out"][0, 2])
        loss += (-w_total / T + PEN * (q_total - JB * K)) / B
    return np.float32(loss)


def run_on_device(preds, y_ts, importance, trace=False, **spmd_kwargs):
    if "nc" not in _CACHE:
        _CACHE["nc"] = _build_program()
    nc = _CACHE["nc"]
    in_maps = _prepare_in_maps(preds, y_ts, importance)
    res = run_bass_kernel_spmd(
        nc, in_maps, list(range(NCORES)), trace=trace, **spmd_kwargs
    )
    return _combine(res.results), res


def kernel(preds, y_ts, importance):
    loss, _ = run_on_device(preds, y_ts, importance, trace=False)
    return loss
